# revision 1
# baseline (speedup 1.0000x reference)
"""Trainium2 Bass kernel for Conv2DCollapse_w_pillar (pillar scatter -> dense BEV).

Strategy ("one-hot matmul scatter"), data-parallel over batch (1 batch / core):
  - Host: dedup pillar rows per flat cell (last write wins, matching the
    reference), sort by cell, bucket into 256-cell blocks, pad each block to K
    rows.  Features are split exactly into 3 bf16 planes (hi/mid/lo) so that
    hi+mid+lo == f32 value bit-exactly.
  - Device: for each pair of blocks, build a one-hot matrix
    oh[i, j] = (cell_id[i] == j) on DVE (is_equal), then 3 accumulating bf16
    matmuls with a block-diagonal stationary operand scatter+transpose the pair
    into PSUM (128 partitions = 2 blocks x 64 channels).  ACT drains PSUM to
    SBUF, big DMAs write the dense (C, ny*nx) plane.  Every output element is
    written exactly once; empty cells get 0 from all-zero one-hot columns.
"""
import sys
sys.path.insert(0, "/opt/trn_rl_repo")
import numpy as np
import ml_dtypes

BF = ml_dtypes.bfloat16
NCORES = 8
C = 64
NX = 512
NY = 512
NXY = NX * NY
BC = 256                 # cells per block
NBLK = NXY // BC         # 1024 blocks per core
NPAIR = NBLK // 2        # 512 pairs per core
CHUNK_PAIRS = 64         # pairs per feature-DMA chunk
NCHUNK = NPAIR // CHUNK_PAIRS
GRP = 8                  # pairs per PSUM group (4 banks)
NSPLIT = 3               # bf16 splits for exact f32

_cache = {}


def _build_nc(K):
    import concourse.bass as bass
    import concourse.tile as tile
    from concourse import bacc, mybir
    from contextlib import ExitStack

    dt = mybir.dt
    K2 = 2 * K
    nc = bacc.Bacc("TRN2", target_bir_lowering=False, debug=False,
                   num_devices=NCORES)
    fe = [nc.dram_tensor(f"fe{s}", [K, NPAIR, C], dt.bfloat16,
                         kind="ExternalInput").ap() for s in range(NSPLIT)]
    fo = [nc.dram_tensor(f"fo{s}", [K, NPAIR, C], dt.bfloat16,
                         kind="ExternalInput").ap() for s in range(NSPLIT)]
    cells_d = nc.dram_tensor("cells", [128, NPAIR], dt.float32,
                             kind="ExternalInput").ap()
    iota_d = nc.dram_tensor("iota", [128, BC], dt.bfloat16,
                            kind="ExternalInput").ap()
    out_d = nc.dram_tensor("out", [C, NXY], dt.float32,
                           kind="ExternalOutput").ap()

    with tile.TileContext(nc) as tc, ExitStack() as ctx:
        const = ctx.enter_context(tc.tile_pool(name="const", bufs=1))
        lhsp = ctx.enter_context(tc.tile_pool(name="lhs", bufs=1))
        ohp = ctx.enter_context(tc.tile_pool(name="oh", bufs=8))
        outp = ctx.enter_context(tc.tile_pool(name="outb", bufs=2))
        psp = ctx.enter_context(tc.tile_pool(name="ps", bufs=2, space="PSUM"))

        iota_t = const.tile([128, BC], dt.bfloat16)
        cells_t = const.tile([128, NPAIR], dt.float32)
        sink = const.tile([128, 2], dt.float32, tag="sink", name="sink")
        nc.gpsimd.dma_start(iota_t[:], iota_d[:])
        nc.gpsimd.dma_start(cells_t[:], cells_d[:])
        # absorber copies: give DVE's clock each preamble-DMA sem one at a time
        # (hardware allows a single embedded sync-wait per instruction)
        nc.vector.tensor_copy(sink[:, 0:1], cells_t[:, 0:1])
        nc.vector.tensor_copy(sink[:, 1:2], iota_t[:, 0:1])

        # persistent zero-stuffed stationary tiles: 2 chunk bufs x NSPLIT
        lhs = [[lhsp.tile([K2, CHUNK_PAIRS * 128], dt.bfloat16,
                          tag=f"lhs{b}_{s}", name=f"lhs{b}_{s}") for s in range(NSPLIT)]
               for b in range(2)]
        for b in range(2):
            for s in range(NSPLIT):
                nc.vector.memset(lhs[b][s][:], 0.0)
        # preamble PE absorber: one ldweights whose wait covers all memsets
        # (single DVE sem lane), so per-chunk absorbers only wait on DMAs
        nc.tensor.ldweights(lhs[1][NSPLIT - 1][0:K, 0:128])

        for c in range(NCHUNK):
            buf = c % 2
            p0 = c * CHUNK_PAIRS
            for s in range(NSPLIT):
                t = lhs[buf][s]
                # even blocks -> rows 0:K, col range [pair*128, pair*128+64)
                dst_e = t[0:K, :].rearrange("k (p f) -> k p f", f=128)[:, :, 0:C]
                nc.sync.dma_start(dst_e, fe[s][:, p0:p0 + CHUNK_PAIRS, :])
                # odd blocks -> rows K:2K, col range [pair*128+64, pair*128+128)
                dst_o = t[K:K2, :].rearrange("k (p f) -> k p f", f=128)[:, :, C:128]
                nc.sync.dma_start(dst_o, fo[s][:, p0:p0 + CHUNK_PAIRS, :])
            for s in range(NSPLIT):
                # absorber: consume the even-DMA sem so real matmuls only
                # need the odd-DMA sem (1-wait limit per instruction)
                nc.tensor.ldweights(lhs[buf][s][0:K, 0:128])
            for g in range(CHUNK_PAIRS // GRP):
                if g % 2 == 0:
                    outb = outp.tile([128, 2 * GRP * BC], dt.float32)
                ps_t = psp.tile([128, GRP * BC], dt.float32)
                for i in range(GRP):
                    p = p0 + g * GRP + i
                    oh = ohp.tile([K2, BC], dt.bfloat16)
                    nc.vector.tensor_scalar(
                        oh[:], iota_t[0:K2, :], cells_t[0:K2, p:p + 1], None,
                        mybir.AluOpType.is_equal)
                    sl = g * GRP + i
                    for s in range(NSPLIT):
                        nc.tensor.matmul(
                            ps_t[:, i * BC:(i + 1) * BC],
                            lhs[buf][s][0:K2, sl * 128:(sl + 1) * 128],
                            oh[:],
                            start=(s == 0), stop=(s == NSPLIT - 1))
                half = (g % 2) * GRP * BC
                nc.scalar.copy(outb[:, half:half + GRP * BC], ps_t[:])
                if g % 2 == 1:
                    base = (p0 + (g - 1) * GRP) * 2 * BC
                    dst4 = out_d[:, base:base + 2 * GRP * 2 * BC].rearrange(
                        "c (p q r) -> c p q r", p=2 * GRP, q=2, r=BC)
                    src_e = outb[0:C, :].rearrange("c (p r) -> c p r", r=BC)
                    src_o = outb[C:128, :].rearrange("c (p r) -> c p r", r=BC)
                    nc.scalar.dma_start(dst4[:, :, 0, :], src_e)
                    nc.scalar.dma_start(dst4[:, :, 1, :], src_o)
    nc.compile()
    return nc


def _prep_core(pf, cell, src, K):
    """pf: (Nb, C) f32 features for this batch (deduped, sorted by cell);
    cell: (Nb,) int cell ids; src unused (rows already gathered)."""
    n = len(cell)
    block = cell // BC
    local = (cell % BC).astype(np.float32)
    starts = np.searchsorted(block, np.arange(NBLK))
    k = np.arange(n) - starts[block]
    assert k.max(initial=0) < K
    pair = block // 2
    parity = block % 2

    x = pf
    hi = x.astype(BF)
    r1 = x - hi.astype(np.float32)
    mid = r1.astype(BF)
    r2 = r1 - mid.astype(np.float32)
    lo = r2.astype(BF)
    assert np.array_equal(
        hi.astype(np.float32) + mid.astype(np.float32) + lo.astype(np.float32), x)
    splits = (hi, mid, lo)

    ev = parity == 0
    od = ~ev
    fe = [np.zeros((K, NPAIR, C), dtype=BF) for _ in range(NSPLIT)]
    fo = [np.zeros((K, NPAIR, C), dtype=BF) for _ in range(NSPLIT)]
    for s in range(NSPLIT):
        fe[s][k[ev], pair[ev], :] = splits[s][ev]
        fo[s][k[od], pair[od], :] = splits[s][od]
    cells = np.full((128, NPAIR), -1.0, np.float32)
    cells[k[ev], pair[ev]] = local[ev]
    cells[K + k[od], pair[od]] = local[od]
    m = {f"fe{s}": fe[s] for s in range(NSPLIT)}
    m.update({f"fo{s}": fo[s] for s in range(NSPLIT)})
    m["cells"] = cells
    m["iota"] = np.broadcast_to(
        np.arange(BC, dtype=np.float32), (128, BC)).astype(BF).copy()
    return m


def kernel(pillar_features, coords, batch_size, nx, ny, num_bev_features,
           **_ignored):
    from concourse import bass_utils

    pf = np.ascontiguousarray(np.asarray(pillar_features, dtype=np.float32))
    co = np.asarray(coords).astype(np.int64)
    B = int(batch_size)
    nx_i, ny_i, C_i = int(nx), int(ny), int(num_bev_features)
    assert (B, nx_i, ny_i, C_i) == (NCORES, NX, NY, C), "hardcoded shape mismatch"

    key = co[:, 0] * NXY + co[:, 1] + co[:, 2] * NX + co[:, 3]
    # dedup, last occurrence wins (matches reference .at[].set semantics)
    n = len(key)
    u, first_rev = np.unique(key[::-1], return_index=True)
    src = n - 1 - first_rev           # original row index that survives
    # u is sorted by (batch, cell)
    batch = (u // NXY).astype(np.int64)
    cell = (u % NXY).astype(np.int64)
    bstart = np.searchsorted(batch, np.arange(NCORES + 1))

    # K: max rows in any 256-cell block, rounded up (shared by all cores)
    blk_global = u // BC
    occ = np.bincount(blk_global - blk_global.min(initial=0)) if len(u) else [0]
    Kmax = int(np.max(np.bincount(blk_global, minlength=1))) if len(u) else 1
    K = max(8, -(-Kmax // 8) * 8)
    assert K <= 64, f"block occupancy {Kmax} too high for pair kernel"

    if K not in _cache:
        _cache[K] = _build_nc(K)
    nc = _cache[K]

    in_maps = []
    for b in range(NCORES):
        lo_i, hi_i = bstart[b], bstart[b + 1]
        in_maps.append(_prep_core(pf[src[lo_i:hi_i]], cell[lo_i:hi_i],
                                  None, K))

    import os
    trace = bool(os.environ.get("BASS_TRACE"))
    res = bass_utils.run_bass_kernel_spmd(
        nc, in_maps, core_ids=list(range(NCORES)), trace=trace)
    kernel._last_results = res

    out = np.empty((NCORES, C, NY, NX), dtype=np.float32)
    for b in range(NCORES):
        out[b] = res.results[b]["out"].reshape(C, NY, NX)
    return out



# revision 6
# speedup vs baseline: 1.1647x; 1.1647x over previous
"""Trainium2 Bass kernel for Conv2DCollapse_w_pillar (pillar scatter -> dense BEV).

Strategy ("one-hot matmul scatter"), data-parallel over batch (1 batch / core):
  - Host: dedup pillar rows per flat cell (last write wins, matching the
    reference), sort by cell, bucket into 256-cell blocks, pad each block to K
    rows.  Features are rounded to a single bf16 plane (rel err ~1e-3, well
    under the 2e-2 gate), packed contiguously as fe/fo = even/odd block rows.
  - Device: features upload as two contiguous DMAs into one [128, NPAIR*C]
    SBUF tile (even rows on partitions 0:K, odd on 64:64+K — engine APs need
    32-aligned partition starts).  DVE expands each chunk into the
    block-diagonal stationary layout [128, 64 pairs x 128] (even rows ->
    cols 0:64 of each 128-lane, odd rows -> cols 64:128).
    For each pair of blocks, DVE builds a one-hot oh[i, j] = (cell_id[i]==j),
    one bf16 matmul scatters+transposes the pair into PSUM (128 partitions =
    2 blocks x 64 channels).  ACT drains PSUM to SBUF, big DMAs write the
    dense (C, ny*nx) plane.  Every output element is written exactly once;
    empty cells get 0 from all-zero one-hot columns.
"""
import sys
sys.path.insert(0, "/opt/trn_rl_repo")
import numpy as np
import ml_dtypes

BF = ml_dtypes.bfloat16
NCORES = 8
C = 64
NX = 512
NY = 512
NXY = NX * NY
BC = 256                 # cells per block
NBLK = NXY // BC         # 1024 blocks per core
NPAIR = NBLK // 2        # 512 pairs per core
CHUNK_PAIRS = 64         # pairs per feature-expansion chunk
NCHUNK = NPAIR // CHUNK_PAIRS
GRP = 8                  # pairs per PSUM group (4 banks)
P = 128                  # partition rows: even rows at 0:K, odd at 64:64+K

_cache = {}


def _build_nc(K):
    import concourse.bass as bass
    import concourse.tile as tile
    from concourse import bacc, mybir
    from contextlib import ExitStack

    dt = mybir.dt
    assert K <= 64
    nc = bacc.Bacc("TRN2", target_bir_lowering=False, debug=False,
                   num_devices=NCORES)
    # packed features: fe = even-block rows, fo = odd-block rows
    fe = nc.dram_tensor("fe", [K, NPAIR, C], dt.bfloat16,
                        kind="ExternalInput").ap()
    fo = nc.dram_tensor("fo", [K, NPAIR, C], dt.bfloat16,
                        kind="ExternalInput").ap()
    cells_d = nc.dram_tensor("cells", [P, NPAIR], dt.float32,
                             kind="ExternalInput").ap()
    iota_d = nc.dram_tensor("iota", [P, BC], dt.bfloat16,
                            kind="ExternalInput").ap()
    out_d = nc.dram_tensor("out", [C, NXY], dt.float32,
                           kind="ExternalOutput").ap()

    with tile.TileContext(nc) as tc, ExitStack() as ctx:
        const = ctx.enter_context(tc.tile_pool(name="const", bufs=1))
        featp = ctx.enter_context(tc.tile_pool(name="feat", bufs=1))
        lhsp = ctx.enter_context(tc.tile_pool(name="lhs", bufs=1))
        ohp = ctx.enter_context(tc.tile_pool(name="oh", bufs=8))
        outp = ctx.enter_context(tc.tile_pool(name="outb", bufs=3))
        psp = ctx.enter_context(tc.tile_pool(name="ps", bufs=2, space="PSUM"))

        iota_t = const.tile([P, BC], dt.bfloat16)
        cells_t = const.tile([P, NPAIR], dt.float32)
        nc.sync.dma_start(iota_t[:], iota_d[:])
        nc.sync.dma_start(cells_t[:], cells_d[:])

        # packed features: partitions 0:K = even rows, 64:64+K = odd rows
        fb = featp.tile([P, NPAIR * C], dt.bfloat16, tag="fb", name="fb")
        fb3 = fb[:].rearrange("k (p f) -> k p f", f=C)

        # persistent stationary tiles (block-diagonal layout), double-buffered
        lhs = [lhsp.tile([P, CHUNK_PAIRS * 128], dt.bfloat16,
                         tag=f"lhs{b}", name=f"lhs{b}") for b in range(2)]
        # zero once (on idle Pool, off DVE's critical path) the halves the
        # expansion copies never write but whose one-hot rows are active;
        # rows K:64 / 112:128 have cells=-1 -> all-zero one-hot, garbage ok
        for b in range(2):
            z3 = lhs[b][:].rearrange("k (p f) -> k p f", f=128)
            nc.gpsimd.memset(z3[0:K, :, C:128], 0.0)
            nc.gpsimd.memset(z3[64:64 + K, :, 0:C], 0.0)

        for c in range(NCHUNK):
            p0 = c * CHUNK_PAIRS
            # feature upload, chunk-granular so chunk 0 compute starts early
            nc.sync.dma_start(fb3[0:K, p0:p0 + CHUNK_PAIRS, :],
                              fe[:, p0:p0 + CHUNK_PAIRS, :])
            nc.sync.dma_start(fb3[64:64 + K, p0:p0 + CHUNK_PAIRS, :],
                              fo[:, p0:p0 + CHUNK_PAIRS, :])

        for c in range(NCHUNK):
            buf = c % 2
            p0 = c * CHUNK_PAIRS
            t3 = lhs[buf][:].rearrange("k (p f) -> k p f", f=128)
            # expand packed chunk into block-diagonal stationary layout
            nc.vector.tensor_copy(
                t3[0:K, :, 0:C],
                fb3[0:K, p0:p0 + CHUNK_PAIRS, :])
            nc.vector.tensor_copy(
                t3[64:64 + K, :, C:128],
                fb3[64:64 + K, p0:p0 + CHUNK_PAIRS, :])
            for g in range(CHUNK_PAIRS // GRP):
                if g % 2 == 0:
                    outb = outp.tile([128, 2 * GRP * BC], dt.float32)
                ps_t = psp.tile([128, GRP * BC], dt.float32)
                for i in range(GRP):
                    p = p0 + g * GRP + i
                    oh = ohp.tile([P, BC], dt.bfloat16)
                    nc.vector.tensor_scalar(
                        oh[:], iota_t[:], cells_t[:, p:p + 1], None,
                        mybir.AluOpType.is_equal)
                    sl = g * GRP + i
                    nc.tensor.matmul(
                        ps_t[:, i * BC:(i + 1) * BC],
                        lhs[buf][0:P, sl * 128:(sl + 1) * 128],
                        oh[:],
                        start=True, stop=True)
                half = (g % 2) * GRP * BC
                nc.scalar.copy(outb[:, half:half + GRP * BC], ps_t[:])
                if g % 2 == 1:
                    base = (p0 + (g - 1) * GRP) * 2 * BC
                    dst4 = out_d[:, base:base + 2 * GRP * 2 * BC].rearrange(
                        "c (p q r) -> c p q r", p=2 * GRP, q=2, r=BC)
                    src_e = outb[0:C, :].rearrange("c (p r) -> c p r", r=BC)
                    src_o = outb[C:128, :].rearrange("c (p r) -> c p r", r=BC)
                    nc.sync.dma_start(dst4[:, :, 0, :], src_e)
                    nc.sync.dma_start(dst4[:, :, 1, :], src_o)
    nc.compile()
    return nc


def _prep_core(pf, cell, K):
    """pf: (Nb, C) f32 features for this batch (deduped, sorted by cell);
    cell: (Nb,) int cell ids."""
    n = len(cell)
    block = cell // BC
    local = (cell % BC).astype(np.float32)
    starts = np.searchsorted(block, np.arange(NBLK))
    k = np.arange(n) - starts[block]
    assert k.max(initial=0) < K
    pair = block // 2
    parity = block % 2

    feat = pf.astype(BF)

    ev = parity == 0
    od = ~ev
    fe = np.zeros((K, NPAIR, C), dtype=BF)
    fo = np.zeros((K, NPAIR, C), dtype=BF)
    fe[k[ev], pair[ev], :] = feat[ev]
    fo[k[od], pair[od], :] = feat[od]
    cells = np.full((P, NPAIR), -1.0, np.float32)
    cells[k[ev], pair[ev]] = local[ev]
    cells[64 + k[od], pair[od]] = local[od]
    return {
        "fe": fe,
        "fo": fo,
        "cells": cells,
        "iota": np.broadcast_to(
            np.arange(BC, dtype=np.float32), (P, BC)).astype(BF).copy(),
    }


def kernel(pillar_features, coords, batch_size, nx, ny, num_bev_features,
           **_ignored):
    from concourse import bass_utils

    pf = np.ascontiguousarray(np.asarray(pillar_features, dtype=np.float32))
    co = np.asarray(coords).astype(np.int64)
    B = int(batch_size)
    nx_i, ny_i, C_i = int(nx), int(ny), int(num_bev_features)
    assert (B, nx_i, ny_i, C_i) == (NCORES, NX, NY, C), "hardcoded shape mismatch"

    key = co[:, 0] * NXY + co[:, 1] + co[:, 2] * NX + co[:, 3]
    # dedup, last occurrence wins (matches reference .at[].set semantics)
    n = len(key)
    u, first_rev = np.unique(key[::-1], return_index=True)
    src = n - 1 - first_rev           # original row index that survives
    # u is sorted by (batch, cell)
    batch = (u // NXY).astype(np.int64)
    cell = (u % NXY).astype(np.int64)
    bstart = np.searchsorted(batch, np.arange(NCORES + 1))

    # K: max rows in any 256-cell block, rounded up (shared by all cores)
    blk_global = u // BC
    Kmax = int(np.max(np.bincount(blk_global, minlength=1))) if len(u) else 1
    K = max(8, -(-Kmax // 8) * 8)
    assert K <= 64, f"block occupancy {Kmax} too high for pair kernel"

    if K not in _cache:
        _cache[K] = _build_nc(K)
    nc = _cache[K]

    in_maps = []
    for b in range(NCORES):
        lo_i, hi_i = bstart[b], bstart[b + 1]
        in_maps.append(_prep_core(pf[src[lo_i:hi_i]], cell[lo_i:hi_i], K))

    import os
    trace = bool(os.environ.get("BASS_TRACE"))
    res = bass_utils.run_bass_kernel_spmd(
        nc, in_maps, core_ids=list(range(NCORES)), trace=trace)
    kernel._last_results = res

    out = np.empty((NCORES, C, NY, NX), dtype=np.float32)
    for b in range(NCORES):
        out[b] = res.results[b]["out"].reshape(C, NY, NX)
    return out
